# revision 41
# baseline (speedup 1.0000x reference)
"""ANIMAZero recurrent cell on 8 TRN2 NeuronCores (Bass/Tile), v3.3.

Data-parallel: batch 1024 -> 8 cores x 128, each core running two
phase-offset chains of 64 batch columns through the T=256 recurrence.

v3 moves part of the sigma/tanh work off the ACT engine onto custom DVE
ops (clamped odd-polynomial approximations, fused with their gating
products), which relieves both the ACT fixed cost (222-cycle SBUF
access) and the two chains' ACT collisions that dominated the old
critical path:

  gmt  = SIG2(p0m)         ~ 2*sigma(mW_pre)        (waw carries the 0.5)
  rI2  = SIGMUL3(p1_r, I)  ~ 2*sigma(r_pre) * I     (wc hI rows carry 0.5)
  hm   = TANHMUL3(p2h, mI) ~ tanh(h_pre) * mI

Hardware constraints found the hard way: a DVE instruction reads at
most ONE tensor input from PSUM, and custom-DVE ops only work with ALL
operands at partition base 0. Hence the stack layout [I; W; A; ones]
(the I slot at base 0 takes the in-place custom writes), p2h/p1_r/p0m
at base 0 of their banks, and W/z/mI/attn/mA/A_new staying exact on
ACT. All biases ride the stack's ones row through the matmuls; clamp
scales are folded into the fp16 weights host-side. Polynomial
coefficients are minimax-fit against this model's empirical
pre-activation distributions (end-to-end rel err <1e-2 in simulation,
gate 2e-2).
"""

import os
import sys

sys.path.insert(0, "/opt/trn_rl_repo")
import numpy as np
import bass_rust
import concourse.bass as bass
import concourse.tile as tile
from concourse import mybir

F32 = mybir.dt.float32
F16 = mybir.dt.float16
SIG = mybir.ActivationFunctionType.Sigmoid
TANH = mybir.ActivationFunctionType.Tanh
MULT = mybir.AluOpType.mult
ADD = mybir.AluOpType.add
SUB = mybir.AluOpType.subtract

D, S, O, T, B = 32, 8, 4, 256, 1024
N_CORES = 8
BC = B // N_CORES  # 128 batch per core
G = BC
WDT = np.float16

# ---------------------------------------------------------------------------
# Polynomial approximations (t-space: t = clip(x/B, -1, 1); sigma gates
# approximate tanh(u), u = x/2). Empirically fit for this model.
PC = {
    "mW": ([0.42091229180656337, -0.023364568802614262], 0.4210716568781921),
    "r": ([0.7540762322323616, -0.11922203245848911], 0.7566058849051597),
    "h": ([1.7090931854219884, -0.8615856160238022], 1.7964036511415415),
}
B_mW, B_r, B_h = (PC[k][1] for k in ("mW", "r", "h"))

# ---------------------------------------------------------------------------
# Custom DVE ops.
from concourse.dve_spec import (  # noqa: E402
    Spec, Src0, Src1, C0, C1, C2, Zero, One, maxx, minn, sq,
    lower as dve_lower, _has_src1,
)
from concourse import dve_ops as _dvo  # noqa: E402
from concourse.dve_uop import DveOpSpec  # noqa: E402


def _register(name, body, reference):
    for o in _dvo.OPS:
        if o.name == name:
            return o
    spec = Spec(body=body, reference=reference)
    row = _dvo._CUSTOM_DVE_ROW_BASE + len(_dvo.OPS)
    assert row < 0x20
    shas = {}
    for ver in ("v3", "v4"):
        s = DveOpSpec(name=name, opcode=row, uops=dve_lower(spec, ver=ver),
                      rd1_en=_has_src1(spec))
        shas[ver] = s.sha(ver)
    op = _dvo.DveOp(name, spec, False, shas)
    _dvo.OPS.append(op)
    _dvo._SUB_OPCODE_FOR_NAME[name] = row
    _dvo.CUSTOM_DVE_SPECS[name] = spec
    return op


def _mk_ops():
    t = maxx(minn(Src0, One), Zero - One)
    t2 = sq(t)

    def np_p3(x, c0, c1):
        tt = np.clip(x, -1, 1)
        return tt * (c0 + c1 * tt * tt)

    sigmul3 = _register(
        "SIGMUL3_ANT", (One + t * (C0 + t2 * C1)) * Src1,
        lambda in0, in1, s0, s1, imm2: (1.0 + np_p3(in0, s0, s1)) * in1,
    )
    sig2 = _register(
        "SIG2_ANT", One + t * (C0 + t2 * C1),
        lambda in0, in1, s0, s1, imm2: 1.0 + np_p3(in0, s0, s1),
    )
    tanhmul3 = _register(
        "TANHMUL3_ANT", (t * (C0 + t2 * C1)) * Src1,
        lambda in0, in1, s0, s1, imm2: np_p3(in0, s0, s1) * in1,
    )
    return sigmul3, sig2, tanhmul3


SIGMUL3_OP, SIG2_OP, TANHMUL3_OP = _mk_ops()

# ---------------------------------------------------------------------------
# walrus in this container rejects instructions carrying more than one sem
# wait ("Too many sync wait commands"). After Tile lowers everything, move
# surplus waits onto same-engine NOPs inserted just before each offender.
_MAXW = 1


def _split_waits(nc):
    for f in nc.m.functions:
        for blk in f.blocks:
            il = blk.instructions
            cur = list(il)
            out_list = []
            changed = False
            for ins in cur:
                si = ins.sync_info
                w = list(si.on_wait or []) if si is not None else []
                if len(w) > _MAXW:
                    changed = True
                    for i in range(0, len(w) - _MAXW, _MAXW):
                        bi = nc.engines[ins.engine].nop(nofuse=True)
                        nop_ins = bi.ins
                        for srch in (blk,) + tuple(f.blocks):
                            lst = srch.instructions
                            if lst and lst[-1] is nop_ins:
                                lst.pop()
                                break
                        nop_ins.sync_info = bass_rust.SyncInfo(
                            on_wait=w[i : i + _MAXW], on_update=[]
                        )
                        out_list.append(nop_ins)
                    si.on_wait = w[len(w) - _MAXW :]
                out_list.append(ins)
            if changed:
                il[:] = out_list


_orig_drain = tile.TileContext._drain_and_barrier


def _drain_then_split(self, tick_clock, wait_clock):
    _orig_drain(self, tick_clock, wait_clock)
    _split_waits(self.nc)


tile.TileContext._drain_and_barrier = _drain_then_split

# ---------------------------------------------------------------------------
# Weight packing. Stack layout: rows 0:32 I, 32:64 W, 64:96 A, row 96 ones.
# All biases ride the ones row; approximation scales folded per-column.

WEIGHT_SPECS = [
    ("wam", [97, 32], F16),  # cols: mW_pre * 1/(2*B_mW)  (SIG2 approx)
    ("waw", [97, 32], F16),  # cols: W_all * 0.5  (gmt = 2*sigma)
    ("wb", [97, 96], F16),   # cols: r * 1/(2*B_r) | z | mI
    ("wc", [97, 32], F16),   # rows: hI*0.5 | hW | hA, all * 1/B_h; h_b row
    ("wdm", [97, 64], F16),  # cols: attn | mA (both exact ACT sigma)
    ("wda", [97, 32], F16),  # cols: A_all; a_b row
    ("wphi", [97, 4], F16),  # rows 64:96 = phi_w.T; row 96 = phi_b
    ("wenc", [32, 128], F16),  # 4x block-diag enc_w.T
    ("biases", [128, 8], F32),
]


def _pack_weights(inp):
    g = {k: np.ascontiguousarray(np.asarray(v, np.float32)) for k, v in inp.items()}

    def stack97(Ip, Wp, Ap, bias, scale=1.0):
        cols = next(m.shape[1] for m in (Ip, Wp, Ap) if m is not None)
        m = np.zeros((97, cols), np.float32)
        if Ip is not None:
            m[0:32] = Ip
        if Wp is not None:
            m[32:64] = Wp
        if Ap is not None:
            m[64:96] = Ap
        if bias is not None:
            m[96] = bias
        return m * scale

    # mW gate contracts [I; A] (wmg over concat(I, A))
    wam = stack97(g["wmg_w"][:, 0:32].T, None, g["wmg_w"][:, 32:64].T,
                  g["wmg_b"], 1.0 / (2.0 * B_mW))
    waw = stack97(g["wI"].T, g["wW"].T, g["wA"].T, None, 0.5)

    rc = stack97(g["rI"].T, g["rW"].T, g["rA"].T, g["r_b"], 1.0 / (2.0 * B_r))
    zc = stack97(g["zI"].T, g["zW"].T, g["zA"].T, g["z_b"])
    # mI gate contracts [W_new; A]
    mic = stack97(None, g["img_w"][:, 0:32].T, g["img_w"][:, 32:64].T, g["img_b"])
    wb = np.concatenate([rc, zc, mic], axis=1)

    wc = stack97(0.5 * g["hI"].T, g["hW"].T, g["hA"].T, g["h_b"], 1.0 / B_h)

    # attn contracts [W; I]; mA contracts [W_new; I_new]
    attc = stack97(g["att_w"][:, 32:64].T, g["att_w"][:, 0:32].T, None, g["att_b"])
    mac = stack97(g["amg_w"][:, 32:64].T, g["amg_w"][:, 0:32].T, None, g["amg_b"])
    wdm = np.concatenate([attc, mac], axis=1)

    wda = stack97(g["aI"].T, g["aW"].T, g["aA"].T, g["a_b"])

    wphi = np.zeros((97, 4), np.float32)
    wphi[64:96] = g["phi_w"].T
    wphi[96] = g["phi_b"]

    wenc = np.zeros((32, 128), np.float32)
    for k in range(4):
        wenc[k * 8 : (k + 1) * 8, k * 32 : (k + 1) * 32] = g["enc_w"].T

    biases = np.zeros((128, 8), np.float32)
    biases[0:32, 4] = g["att_b"]  # t=0 attn bootstrap
    biases[:, 5] = np.tile(g["enc_b"], 4)

    w = dict(wam=wam, waw=waw, wb=wb, wc=wc, wdm=wdm, wda=wda, wphi=wphi,
             wenc=wenc, biases=biases)
    return {
        k: np.ascontiguousarray(v if k == "biases" else v.astype(WDT))
        for k, v in w.items()
    }


def _pack_obs_shard(obs_shard):
    """[T, BC, S] f32 -> [32, T/4*BC] fp16: row k*8+s, col c*BC+b holds
    obs[4c+k, b, s] (4 timesteps stacked on partitions)."""
    x = np.ascontiguousarray(obs_shard).reshape(T // 4, 4, BC, S)
    x = x.transpose(1, 3, 0, 2)
    return np.ascontiguousarray(x.reshape(32, (T // 4) * BC)).astype(WDT)


def _unpack_out(out_core):
    """[4, T/4, 4, BC] -> [T, BC, O]."""
    return np.ascontiguousarray(
        np.asarray(out_core).reshape(O, T, BC).transpose(1, 2, 0)
    )


def _build_nc():
    nc = bass.Bass()
    obs4 = nc.declare_dram_parameter("obs4", [32, (T // 4) * BC], F16, isOutput=False)
    wdram = {}
    for name, shape, dt in WEIGHT_SPECS:
        wdram[name] = nc.declare_dram_parameter(name, shape, dt, isOutput=False)
    out = nc.declare_dram_parameter("out", [4, T // 4, 4, BC], F32, isOutput=True)

    NCH = 2
    G2 = G // NCH
    c_mW = PC["mW"][0]
    c_r = PC["r"][0]
    c_h = PC["h"][0]

    with tile.TileContext(nc) as tc:
        with (
            tc.tile_pool(name="singles", bufs=1) as singles,
            tc.tile_pool(name="psum", bufs=1, space="PSUM") as psum,
            tc.tile_pool(name="outp", bufs=3) as outp,
        ):
            wsb = {}
            for name, shape, dt in WEIGHT_SPECS:
                wsb[name] = singles.tile(shape, dt, name=f"w_{name}")
                nc.sync.dma_start(out=wsb[name], in_=wdram[name][:, :])
            obs_sb = singles.tile([32, (T // 4) * BC], F16)
            nc.sync.dma_start(out=obs_sb, in_=obs4[:, :])

            bia = wsb["biases"]

            # obs_enc_all = tanh(wenc.T @ obs4 + enc_b), all steps up front
            oenc = singles.tile([128, (T // 4) * BC], F16)
            NPRE = (T // 4) * BC // 512
            with tc.tile_pool(name="psum_pre", bufs=1, space="PSUM") as psum_pre:
                for i in range(NPRE):
                    ppre = psum_pre.tile([128, 512], F32)
                    nc.tensor.matmul(
                        ppre, wsb["wenc"], obs_sb[:, i * 512 : (i + 1) * 512],
                        start=True, stop=True,
                    )
                    nc.scalar.activation(
                        out=oenc[:, i * 512 : (i + 1) * 512], in_=ppre,
                        func=TANH, bias=bia[:, 5:6],
                    )

            def chain_tiles(h):
                d = {}
                s = lambda nm, shape: singles.tile(shape, F16, name=f"{nm}_{h}")
                d["icp2"] = s("icp2", [32, G2])  # I snapshot (Pool-written)
                d["gmt"] = s("gmt", [32, G2])    # 2*sigma(mW)  (DVE SIG2)
                d["zt"] = [s(f"zt{i}", [32, G2]) for i in range(2)]
                d["mit"] = [s(f"mit{i}", [32, G2]) for i in range(2)]  # sig(mI) @0
                d["mat"] = [s(f"mat{i}", [64, G2]) for i in range(2)]  # sig(mA) @32
                d["att"] = s("att", [128, G2])  # attn at k2*32
                d["hmt"] = s("hmt", [32, G2])
                d["vt"] = s("vt", [32, G2])
                d["un"] = [s(f"un{i}", [32, G2]) for i in range(2)]
                d["xat"] = s("xat", [128, G2])
                d["xbt"] = s("xbt", [128, G2])
                d["wpt"] = [s(f"wpt{i}", [32, G2]) for i in range(2)]
                d["stk"] = s("stk", [128, G2])
                nc.vector.memset(d["stk"], 0.0)
                nc.vector.memset(d["stk"][96:97, :], 1.0)  # ones row
                nc.vector.memset(d["icp2"], 0.0)  # I(-1) = 0
                nc.vector.memset(d["xbt"], 0.0)  # xb(0) = 0
                return d

            CH = [chain_tiles(h) for h in range(NCH)]
            for h in range(NCH):
                d = CH[h]
                tA = psum.tile([128, 2 * G2], F32, name=f"psA_{h}")
                tB = psum.tile([128, 2 * G2], F32, name=f"psB_{h}")
                tC = psum.tile([128, 7 * G2], F32, name=f"psC_{h}")
                d["pmx"] = tA[0:64, 0:G2]       # attn-pre @0:32, mA-pre @32:64
                d["p0m"] = tA[0:32, G2 : 2 * G2]
                d["p1"] = tB[0:96, 0:G2]        # r @0:32, z @32:64, mI @64:96
                d["p0w"] = tB[0:32, G2 : 2 * G2]
                d["p2h"] = tC[0:32, 0:G2]
                d["patps"] = [tC[64:96, 0:G2], tC[96:128, 0:G2]]
                d["p3a"] = tC[32:64, G2 : 2 * G2]
                d["p4"] = tC[0:4, 3 * G2 : 7 * G2]
            zz = singles.tile([32, G2], F16, name="zz")
            at0 = singles.tile([32, G2], F16, name="at0")
            nc.vector.memset(zz, 0.0)

            def emit_phi(h, t):
                # phi matmul for step t (phi_b rides the ones row), deferred
                # into step t+1's window.
                d = CH[h]
                nc.tensor.matmul(
                    d["p4"][0:4, (t % 4) * G2 : (t % 4 + 1) * G2],
                    wsb["wphi"][64:97, :], d["stk"][64:97, :],
                    start=True, stop=True,
                )

            def emit_evac(h, t):
                if t < 0 or t % 4 != 3:
                    return
                d = CH[h]
                ch = outp.tile([4, 4 * G2], F32)
                nc.vector.tensor_copy(out=ch, in_=d["p4"])
                nc.sync.dma_start(
                    out=out[0:4, t // 4, 0:4, h * G2 : (h + 1) * G2], in_=ch
                )

            def oe_slice(h, t):
                c, k = t // 4, t % 4
                return oenc[
                    k * 32 : (k + 1) * 32,
                    c * BC + h * G2 : c * BC + (h + 1) * G2,
                ]

            # t=0 bootstrap: attn(0) = sigmoid(att_b); xa0 = attn0*oe0
            nc.scalar.activation(out=at0, in_=zz, func=SIG, bias=bia[0:32, 4:5])
            for h in range(NCH):
                nc.gpsimd.tensor_tensor(
                    out=CH[h]["xat"][0:32, :], in0=at0, in1=oe_slice(h, 0), op=MULT,
                )

            def emit_half(c, t, o, s):
                """One half-period: chain c runs its B stage for step t,
                interleaved (in ideal-schedule time order per engine) with
                chain o's C stage for step s and A stage for step s+1.
                Skip o's parts when s < 0 (warmup)."""
                dc, do = CH[c], CH[o]
                bt = t % 2
                bs = s % 2
                k2 = (s + 1) % 4
                ks = (s + 1) % 4

                # -- c: W_new(t) = tanh(wpre) (wpre built last half)
                nc.scalar.activation(out=dc["stk"][32:64, :], in_=dc["wpt"][bt], func=TANH)
                if s >= 0:
                    # -- o: C-stage matmuls for step s
                    s97o = do["stk"][0:97, :]
                    nc.tensor.matmul(do["pmx"], wsb["wdm"], s97o, start=True, stop=True)
                    nc.tensor.matmul(do["p3a"], wsb["wda"], s97o, start=True,
                                     stop=True, tile_position=(0, 32))
                    if s + 1 < T:
                        # I(s) snapshot early on Pool (ready at half start,
                        # ahead of xat in the Pool FIFO)
                        nc.gpsimd.tensor_copy(out=do["icp2"], in_=do["stk"][0:32, :])
                    nc.scalar.activation(out=do["mat"][bs][32:64, :],
                                         in_=do["pmx"][32:64, :], func=SIG)
                    nc.vector.tensor_tensor(out=do["patps"][bs],
                                            in0=do["mat"][bs][32:64, :],
                                            in1=do["p3a"], op=MULT)
                    if s + 1 < T:
                        att = do["att"][k2 * 32 : k2 * 32 + 32, :]
                        nc.scalar.activation(out=att, in_=do["pmx"][0:32, :], func=SIG)
                # -- c: B stage for step t
                nc.tensor.matmul(dc["p1"], wsb["wb"], dc["stk"][0:97, :], start=True, stop=True)
                nc.scalar.activation(out=dc["zt"][bt], in_=dc["p1"][32:64, :], func=SIG)
                # rI2 = 2*sig(r)*I overwriting the I slot (reads the Pool
                # snapshot, so not in-place); wc hI rows carry the 0.5
                nc.vector._custom_dve(
                    SIGMUL3_OP, out=dc["stk"][0:32, :], in0=dc["p1"][0:32, :],
                    in1=dc["icp2"], s0=c_r[0], s1=c_r[1],
                )
                # un = (z-1)*I in one fused DVE op (off the critical path
                # here; Pool's FIFO stalled it badly)
                nc.vector.scalar_tensor_tensor(
                    out=dc["un"][bt], in0=dc["zt"][bt], scalar=1.0, in1=dc["icp2"],
                    op0=SUB, op1=MULT,
                )
                if s >= 0 and s + 1 < T:
                    nc.gpsimd.tensor_tensor(
                        out=do["xat"][k2 * 32 : k2 * 32 + 32, :],
                        in0=do["att"][k2 * 32 : k2 * 32 + 32, :],
                        in1=oe_slice(o, s + 1), op=MULT,
                    )
                if s >= 0:
                    # -- o: A_new(s) = tanh(pat)
                    nc.scalar.activation(out=do["stk"][64:96, :], in_=do["patps"][bs], func=TANH)
                nc.tensor.matmul(dc["p2h"], wsb["wc"], dc["stk"][0:97, :], start=True,
                                 stop=True, tile_position=(0, 0))
                nc.scalar.activation(out=dc["mit"][bt], in_=dc["p1"][64:96, :], func=SIG)
                # hm = tanh3(h_pre) * mI
                nc.vector._custom_dve(
                    TANHMUL3_OP, out=dc["hmt"], in0=dc["p2h"], in1=dc["mit"][bt],
                    s0=c_h[0], s1=c_h[1],
                )
                # -- o: A stage for step s+1
                if s + 1 < T and s >= 0:
                    s97o = do["stk"][0:97, :]
                    nc.tensor.matmul(do["p0m"], wsb["wam"], s97o, start=True, stop=True)
                    nc.tensor.matmul(do["p0w"], wsb["waw"], s97o, start=True,
                                     stop=True, tile_position=(0, 0))
                    emit_phi(o, s)
                    nc.vector._custom_dve(
                        SIG2_OP, out=do["gmt"], in0=do["p0m"], s0=c_mW[0], s1=c_mW[1],
                    )
                nc.gpsimd.tensor_tensor(out=dc["vt"], in0=dc["zt"][bt], in1=dc["hmt"], op=MULT)
                # I_new = v - (z-1)*I  (before xb_o: unblocks wdm -> sigma-mA)
                nc.vector.tensor_tensor(out=dc["stk"][0:32, :], in0=dc["vt"], in1=dc["un"][bt], op=SUB)
                if s + 1 < T and s >= 0:
                    xbo = do["xbt"][ks * 32 : ks * 32 + 32, :]
                    nc.vector.tensor_tensor(out=xbo, in0=do["gmt"], in1=do["p0w"], op=MULT)
                if s + 1 < T:
                    xao = do["xat"][ks * 32 : ks * 32 + 32, :]
                    xbo = do["xbt"][ks * 32 : ks * 32 + 32, :]
                    nc.vector.tensor_tensor(out=do["wpt"][(s + 1) % 2], in0=xao,
                                            in1=xbo, op=ADD)
                emit_evac(c, t - 1)

            # Warmup: both chains' step-0 W-branch inputs (xb=0, xa=attn0*oe0
            # from the bootstrap), then the steady half-period interleave.
            for h in range(NCH):
                nc.vector.tensor_tensor(out=CH[h]["wpt"][0], in0=CH[h]["xat"][0:32, :],
                                        in1=CH[h]["xbt"][0:32, :], op=ADD)
            for t in range(T):
                emit_half(0, t, 1, t - 1)
                emit_half(1, t, 0, t)
            # Tail: chain 1's C stage for the last step.
            s97o = CH[1]["stk"][0:97, :]
            nc.tensor.matmul(CH[1]["pmx"], wsb["wdm"], s97o, start=True, stop=True)
            nc.tensor.matmul(CH[1]["p3a"], wsb["wda"], s97o, start=True, stop=True,
                             tile_position=(0, 32))
            nc.scalar.activation(out=CH[1]["mat"][(T - 1) % 2][32:64, :],
                                 in_=CH[1]["pmx"][32:64, :], func=SIG)
            nc.vector.tensor_tensor(out=CH[1]["patps"][(T - 1) % 2],
                                    in0=CH[1]["mat"][(T - 1) % 2][32:64, :],
                                    in1=CH[1]["p3a"], op=MULT)
            nc.scalar.activation(out=CH[1]["stk"][64:96, :],
                                 in_=CH[1]["patps"][(T - 1) % 2], func=TANH)
            for h in range(NCH):
                emit_phi(h, T - 1)
            for h in range(NCH):
                emit_evac(h, T - 1)
    # Populate .instr bytes for InstISA subclasses (InstCustomDveAnt) —
    # raw Bass skips this Bacc pass; without it walrus codegen fails with
    # "ISA wrong length".
    mybir.codegen_inst_isa_subclasses(nc)
    return nc


_NC_CACHE = None


def kernel(**inputs):
    global _NC_CACHE
    from concourse.bass_utils import run_bass_kernel_spmd

    obs = np.ascontiguousarray(np.asarray(inputs["obs"], np.float32))
    w = _pack_weights({k: v for k, v in inputs.items() if k != "obs"})

    if _NC_CACHE is None:
        _NC_CACHE = _build_nc()
    nc = _NC_CACHE

    in_maps = []
    for i in range(N_CORES):
        m = dict(w)
        m["obs4"] = _pack_obs_shard(obs[:, i * BC : (i + 1) * BC, :])
        in_maps.append(m)

    res = run_bass_kernel_spmd(
        nc, in_maps, core_ids=list(range(N_CORES)), trace=False
    )
    outs = [_unpack_out(np.asarray(res.results[i]["out"])) for i in range(N_CORES)]
    return np.concatenate(outs, axis=1).astype(np.float32)  # [T, B, O]


# revision 42
# speedup vs baseline: 1.2068x; 1.2068x over previous
"""ANIMAZero recurrent cell on 8 TRN2 NeuronCores (Bass/Tile), v3.3.

Data-parallel: batch 1024 -> 8 cores x 128, each core running two
phase-offset chains of 64 batch columns through the T=256 recurrence.

v3 moves part of the sigma/tanh work off the ACT engine onto custom DVE
ops (clamped odd-polynomial approximations, fused with their gating
products), which relieves both the ACT fixed cost (222-cycle SBUF
access) and the two chains' ACT collisions that dominated the old
critical path:

  gmt  = SIG2(p0m)         ~ 2*sigma(mW_pre)        (waw carries the 0.5)
  rI2  = SIGMUL3(p1_r, I)  ~ 2*sigma(r_pre) * I     (wc hI rows carry 0.5)
  hm   = TANHMUL3(p2h, mI) ~ tanh(h_pre) * mI

Hardware constraints found the hard way: a DVE instruction reads at
most ONE tensor input from PSUM, and custom-DVE ops only work with ALL
operands at partition base 0. Hence the stack layout [I; W; A; ones]
(the I slot at base 0 takes the in-place custom writes), p2h/p1_r/p0m
at base 0 of their banks, and W/z/mI/attn/mA/A_new staying exact on
ACT. All biases ride the stack's ones row through the matmuls; clamp
scales are folded into the fp16 weights host-side. Polynomial
coefficients are minimax-fit against this model's empirical
pre-activation distributions (end-to-end rel err <1e-2 in simulation,
gate 2e-2).
"""

import os
import sys

sys.path.insert(0, "/opt/trn_rl_repo")
import numpy as np
import bass_rust
import concourse.bass as bass
import concourse.tile as tile
from concourse import mybir

F32 = mybir.dt.float32
F16 = mybir.dt.float16
SIG = mybir.ActivationFunctionType.Sigmoid
TANH = mybir.ActivationFunctionType.Tanh
MULT = mybir.AluOpType.mult
ADD = mybir.AluOpType.add
SUB = mybir.AluOpType.subtract

D, S, O, T, B = 32, 8, 4, 256, 1024
N_CORES = 8
BC = B // N_CORES  # 128 batch per core
G = BC
WDT = np.float16

# ---------------------------------------------------------------------------
# Polynomial approximations (t-space: t = clip(x/B, -1, 1); sigma gates
# approximate tanh(u), u = x/2). Empirically fit for this model.
PC = {
    "mW": ([0.42091229180656337, -0.023364568802614262], 0.4210716568781921),
    "r": ([0.7540762322323616, -0.11922203245848911], 0.7566058849051597),
    "h": ([1.7090931854219884, -0.8615856160238022], 1.7964036511415415),
}
B_mW, B_r, B_h = (PC[k][1] for k in ("mW", "r", "h"))

# ---------------------------------------------------------------------------
# Custom DVE ops.
from concourse.dve_spec import (  # noqa: E402
    Spec, Src0, Src1, C0, C1, C2, Zero, One, maxx, minn, sq,
    lower as dve_lower, _has_src1,
)
from concourse import dve_ops as _dvo  # noqa: E402
from concourse.dve_uop import DveOpSpec  # noqa: E402


def _register(name, body, reference):
    for o in _dvo.OPS:
        if o.name == name:
            return o
    spec = Spec(body=body, reference=reference)
    row = _dvo._CUSTOM_DVE_ROW_BASE + len(_dvo.OPS)
    assert row < 0x20
    shas = {}
    for ver in ("v3", "v4"):
        s = DveOpSpec(name=name, opcode=row, uops=dve_lower(spec, ver=ver),
                      rd1_en=_has_src1(spec))
        shas[ver] = s.sha(ver)
    op = _dvo.DveOp(name, spec, False, shas)
    _dvo.OPS.append(op)
    _dvo._SUB_OPCODE_FOR_NAME[name] = row
    _dvo.CUSTOM_DVE_SPECS[name] = spec
    return op


def _mk_ops():
    t = maxx(minn(Src0, One), Zero - One)
    t2 = sq(t)

    def np_p3(x, c0, c1):
        tt = np.clip(x, -1, 1)
        return tt * (c0 + c1 * tt * tt)

    sigmul3 = _register(
        "SIGMUL3_ANT", (One + t * (C0 + t2 * C1)) * Src1,
        lambda in0, in1, s0, s1, imm2: (1.0 + np_p3(in0, s0, s1)) * in1,
    )
    sig2 = _register(
        "SIG2_ANT", One + t * (C0 + t2 * C1),
        lambda in0, in1, s0, s1, imm2: 1.0 + np_p3(in0, s0, s1),
    )
    tanhmul3 = _register(
        "TANHMUL3_ANT", (t * (C0 + t2 * C1)) * Src1,
        lambda in0, in1, s0, s1, imm2: np_p3(in0, s0, s1) * in1,
    )
    return sigmul3, sig2, tanhmul3


SIGMUL3_OP, SIG2_OP, TANHMUL3_OP = _mk_ops()

# ---------------------------------------------------------------------------
# walrus in this container rejects instructions carrying more than one sem
# wait ("Too many sync wait commands"). After Tile lowers everything, move
# surplus waits onto same-engine NOPs inserted just before each offender.
_MAXW = 1


def _split_waits(nc):
    for f in nc.m.functions:
        for blk in f.blocks:
            il = blk.instructions
            cur = list(il)
            out_list = []
            changed = False
            for ins in cur:
                si = ins.sync_info
                w = list(si.on_wait or []) if si is not None else []
                if len(w) > _MAXW:
                    changed = True
                    for i in range(0, len(w) - _MAXW, _MAXW):
                        bi = nc.engines[ins.engine].nop(nofuse=True)
                        nop_ins = bi.ins
                        for srch in (blk,) + tuple(f.blocks):
                            lst = srch.instructions
                            if lst and lst[-1] is nop_ins:
                                lst.pop()
                                break
                        nop_ins.sync_info = bass_rust.SyncInfo(
                            on_wait=w[i : i + _MAXW], on_update=[]
                        )
                        out_list.append(nop_ins)
                    si.on_wait = w[len(w) - _MAXW :]
                out_list.append(ins)
            if changed:
                il[:] = out_list


_orig_drain = tile.TileContext._drain_and_barrier


def _drain_then_split(self, tick_clock, wait_clock):
    _orig_drain(self, tick_clock, wait_clock)
    _split_waits(self.nc)


tile.TileContext._drain_and_barrier = _drain_then_split

# ---------------------------------------------------------------------------
# Weight packing. Stack layout: rows 0:32 I, 32:64 W, 64:96 A, row 96 ones.
# All biases ride the ones row; approximation scales folded per-column.

WEIGHT_SPECS = [
    ("wam", [97, 32], F16),  # cols: mW_pre * 1/(2*B_mW)  (SIG2 approx)
    ("waw", [97, 32], F16),  # cols: W_all * 0.5  (gmt = 2*sigma)
    ("wb", [97, 96], F16),   # cols: r * 1/(2*B_r) | z | mI
    ("wc", [97, 32], F16),   # rows: hI*0.5 | hW | hA, all * 1/B_h; h_b row
    ("wdm", [97, 64], F16),  # cols: attn | mA (both exact ACT sigma)
    ("wda", [97, 32], F16),  # cols: A_all; a_b row
    ("wphi", [97, 4], F16),  # rows 64:96 = phi_w.T; row 96 = phi_b
    ("wenc", [32, 128], F16),  # 4x block-diag enc_w.T
    ("biases", [128, 8], F32),
]


def _pack_weights(inp):
    g = {k: np.ascontiguousarray(np.asarray(v, np.float32)) for k, v in inp.items()}

    def stack97(Ip, Wp, Ap, bias, scale=1.0):
        cols = next(m.shape[1] for m in (Ip, Wp, Ap) if m is not None)
        m = np.zeros((97, cols), np.float32)
        if Ip is not None:
            m[0:32] = Ip
        if Wp is not None:
            m[32:64] = Wp
        if Ap is not None:
            m[64:96] = Ap
        if bias is not None:
            m[96] = bias
        return m * scale

    # mW gate contracts [I; A] (wmg over concat(I, A))
    wam = stack97(g["wmg_w"][:, 0:32].T, None, g["wmg_w"][:, 32:64].T,
                  g["wmg_b"], 1.0 / (2.0 * B_mW))
    waw = stack97(g["wI"].T, g["wW"].T, g["wA"].T, None, 0.5)

    rc = stack97(g["rI"].T, g["rW"].T, g["rA"].T, g["r_b"], 1.0 / (2.0 * B_r))
    zc = stack97(g["zI"].T, g["zW"].T, g["zA"].T, g["z_b"])
    # mI gate contracts [W_new; A]
    mic = stack97(None, g["img_w"][:, 0:32].T, g["img_w"][:, 32:64].T, g["img_b"])
    wb = np.concatenate([rc, zc, mic], axis=1)

    wc = stack97(0.5 * g["hI"].T, g["hW"].T, g["hA"].T, g["h_b"], 1.0 / B_h)

    # attn contracts [W; I]; mA contracts [W_new; I_new]
    attc = stack97(g["att_w"][:, 32:64].T, g["att_w"][:, 0:32].T, None, g["att_b"])
    mac = stack97(g["amg_w"][:, 32:64].T, g["amg_w"][:, 0:32].T, None, g["amg_b"])
    wdm = np.concatenate([attc, mac], axis=1)

    wda = stack97(g["aI"].T, g["aW"].T, g["aA"].T, g["a_b"])

    wphi = np.zeros((97, 4), np.float32)
    wphi[64:96] = g["phi_w"].T
    wphi[96] = g["phi_b"]

    wenc = np.zeros((32, 128), np.float32)
    for k in range(4):
        wenc[k * 8 : (k + 1) * 8, k * 32 : (k + 1) * 32] = g["enc_w"].T

    biases = np.zeros((128, 8), np.float32)
    biases[0:32, 4] = g["att_b"]  # t=0 attn bootstrap
    biases[:, 5] = np.tile(g["enc_b"], 4)

    w = dict(wam=wam, waw=waw, wb=wb, wc=wc, wdm=wdm, wda=wda, wphi=wphi,
             wenc=wenc, biases=biases)
    return {
        k: np.ascontiguousarray(v if k == "biases" else v.astype(WDT))
        for k, v in w.items()
    }


def _pack_obs_shard(obs_shard):
    """[T, BC, S] f32 -> [32, T/4*BC] fp16: row k*8+s, col c*BC+b holds
    obs[4c+k, b, s] (4 timesteps stacked on partitions)."""
    x = np.ascontiguousarray(obs_shard).reshape(T // 4, 4, BC, S)
    x = x.transpose(1, 3, 0, 2)
    return np.ascontiguousarray(x.reshape(32, (T // 4) * BC)).astype(WDT)


def _unpack_out(out_core):
    """[4, T/4, 4, BC] -> [T, BC, O]."""
    return np.ascontiguousarray(
        np.asarray(out_core).reshape(O, T, BC).transpose(1, 2, 0)
    )


def _build_nc():
    nc = bass.Bass()
    obs4 = nc.declare_dram_parameter("obs4", [32, (T // 4) * BC], F16, isOutput=False)
    wdram = {}
    for name, shape, dt in WEIGHT_SPECS:
        wdram[name] = nc.declare_dram_parameter(name, shape, dt, isOutput=False)
    out = nc.declare_dram_parameter("out", [4, T // 4, 4, BC], F32, isOutput=True)

    NCH = 2
    G2 = G // NCH
    c_mW = PC["mW"][0]
    c_r = PC["r"][0]
    c_h = PC["h"][0]

    with tile.TileContext(nc) as tc:
        with (
            tc.tile_pool(name="singles", bufs=1) as singles,
            tc.tile_pool(name="psum", bufs=1, space="PSUM") as psum,
            tc.tile_pool(name="outp", bufs=3) as outp,
        ):
            wsb = {}
            for name, shape, dt in WEIGHT_SPECS:
                wsb[name] = singles.tile(shape, dt, name=f"w_{name}")
                nc.sync.dma_start(out=wsb[name], in_=wdram[name][:, :])
            obs_sb = singles.tile([32, (T // 4) * BC], F16)
            nc.sync.dma_start(out=obs_sb, in_=obs4[:, :])

            bia = wsb["biases"]

            # obs_enc_all = tanh(wenc.T @ obs4 + enc_b), all steps up front
            oenc = singles.tile([128, (T // 4) * BC], F16)
            NPRE = (T // 4) * BC // 512
            with tc.tile_pool(name="psum_pre", bufs=1, space="PSUM") as psum_pre:
                for i in range(NPRE):
                    ppre = psum_pre.tile([128, 512], F32)
                    nc.tensor.matmul(
                        ppre, wsb["wenc"], obs_sb[:, i * 512 : (i + 1) * 512],
                        start=True, stop=True,
                    )
                    nc.scalar.activation(
                        out=oenc[:, i * 512 : (i + 1) * 512], in_=ppre,
                        func=TANH, bias=bia[:, 5:6],
                    )

            def chain_tiles(h):
                d = {}
                s = lambda nm, shape: singles.tile(shape, F16, name=f"{nm}_{h}")
                d["icp2"] = s("icp2", [32, G2])  # I snapshot (Pool-written)
                d["gmt"] = s("gmt", [32, G2])    # 2*sigma(mW)  (DVE SIG2)
                d["zt"] = [s(f"zt{i}", [32, G2]) for i in range(2)]
                d["mit"] = [s(f"mit{i}", [32, G2]) for i in range(2)]  # sig(mI) @0
                d["mat"] = [s(f"mat{i}", [64, G2]) for i in range(2)]  # sig(mA) @32
                d["att"] = s("att", [128, G2])  # attn at k2*32
                d["hmt"] = s("hmt", [32, G2])
                d["vt"] = s("vt", [32, G2])
                d["un"] = [s(f"un{i}", [32, G2]) for i in range(2)]
                d["xat"] = s("xat", [128, G2])
                d["xbt"] = s("xbt", [128, G2])
                d["wpt"] = [s(f"wpt{i}", [32, G2]) for i in range(2)]
                d["stk"] = s("stk", [128, G2])
                nc.vector.memset(d["stk"], 0.0)
                nc.vector.memset(d["stk"][96:97, :], 1.0)  # ones row
                nc.vector.memset(d["icp2"], 0.0)  # I(-1) = 0
                nc.vector.memset(d["xbt"], 0.0)  # xb(0) = 0
                return d

            CH = [chain_tiles(h) for h in range(NCH)]
            for h in range(NCH):
                d = CH[h]
                tA = psum.tile([128, 2 * G2], F32, name=f"psA_{h}")
                tB = psum.tile([128, 2 * G2], F32, name=f"psB_{h}")
                tC = psum.tile([128, 7 * G2], F32, name=f"psC_{h}")
                d["pmx"] = tA[0:64, 0:G2]       # attn-pre @0:32, mA-pre @32:64
                d["p0m"] = tA[0:32, G2 : 2 * G2]
                d["p1"] = tB[0:96, 0:G2]        # r @0:32, z @32:64, mI @64:96
                d["p0w"] = tB[0:32, G2 : 2 * G2]
                d["p2h"] = tC[0:32, 0:G2]
                d["patps"] = [tC[64:96, 0:G2], tC[96:128, 0:G2]]
                d["p3a"] = tC[32:64, G2 : 2 * G2]
                d["p4"] = tC[0:4, 3 * G2 : 7 * G2]
            zz = singles.tile([32, G2], F16, name="zz")
            at0 = singles.tile([32, G2], F16, name="at0")
            nc.vector.memset(zz, 0.0)

            def emit_phi(h, t):
                # phi matmul for step t (phi_b rides the ones row), deferred
                # into step t+1's window.
                d = CH[h]
                nc.tensor.matmul(
                    d["p4"][0:4, (t % 4) * G2 : (t % 4 + 1) * G2],
                    wsb["wphi"][64:97, :], d["stk"][64:97, :],
                    start=True, stop=True,
                )

            def emit_evac(h, t):
                if t < 0 or t % 4 != 3:
                    return
                d = CH[h]
                ch = outp.tile([4, 4 * G2], F32)
                nc.vector.tensor_copy(out=ch, in_=d["p4"])
                nc.sync.dma_start(
                    out=out[0:4, t // 4, 0:4, h * G2 : (h + 1) * G2], in_=ch
                )

            def oe_slice(h, t):
                c, k = t // 4, t % 4
                return oenc[
                    k * 32 : (k + 1) * 32,
                    c * BC + h * G2 : c * BC + (h + 1) * G2,
                ]

            # t=0 bootstrap: attn(0) = sigmoid(att_b); xa0 = attn0*oe0
            nc.scalar.activation(out=at0, in_=zz, func=SIG, bias=bia[0:32, 4:5])
            for h in range(NCH):
                nc.gpsimd.tensor_tensor(
                    out=CH[h]["xat"][0:32, :], in0=at0, in1=oe_slice(h, 0), op=MULT,
                )

            def emit_half(c, t, o, s):
                """One half-period: chain c runs its B stage for step t,
                interleaved (in ideal-schedule time order per engine) with
                chain o's C stage for step s and A stage for step s+1.
                Skip o's parts when s < 0 (warmup)."""
                dc, do = CH[c], CH[o]
                bt = t % 2
                bs = s % 2
                k2 = (s + 1) % 4
                ks = (s + 1) % 4

                # -- c: W_new(t) = tanh(wpre) (wpre built last half)
                nc.scalar.activation(out=dc["stk"][32:64, :], in_=dc["wpt"][bt], func=TANH)
                if s >= 0:
                    # -- o: C-stage matmuls for step s
                    s97o = do["stk"][0:97, :]
                    nc.tensor.matmul(do["pmx"], wsb["wdm"], s97o, start=True, stop=True)
                    nc.tensor.matmul(do["p3a"], wsb["wda"], s97o, start=True,
                                     stop=True, tile_position=(0, 32))
                    if s + 1 < T:
                        # I(s) snapshot early on Pool (ready at half start,
                        # ahead of xat in the Pool FIFO)
                        nc.gpsimd.tensor_copy(out=do["icp2"], in_=do["stk"][0:32, :])
                    nc.scalar.activation(out=do["mat"][bs][32:64, :],
                                         in_=do["pmx"][32:64, :], func=SIG)
                    nc.vector.tensor_tensor(out=do["patps"][bs],
                                            in0=do["mat"][bs][32:64, :],
                                            in1=do["p3a"], op=MULT)
                    if s + 1 < T:
                        att = do["att"][k2 * 32 : k2 * 32 + 32, :]
                        nc.scalar.activation(out=att, in_=do["pmx"][0:32, :], func=SIG)
                # -- c: B stage for step t
                nc.tensor.matmul(dc["p1"], wsb["wb"], dc["stk"][0:97, :], start=True, stop=True)
                nc.scalar.activation(out=dc["zt"][bt], in_=dc["p1"][32:64, :], func=SIG)
                # rI2 = 2*sig(r)*I overwriting the I slot (reads the Pool
                # snapshot, so not in-place); wc hI rows carry the 0.5
                nc.vector._custom_dve(
                    SIGMUL3_OP, out=dc["stk"][0:32, :], in0=dc["p1"][0:32, :],
                    in1=dc["icp2"], s0=c_r[0], s1=c_r[1],
                )
                # un = (z-1)*I in one fused DVE op (off the critical path
                # here; Pool's FIFO stalled it badly)
                nc.vector.scalar_tensor_tensor(
                    out=dc["un"][bt], in0=dc["zt"][bt], scalar=1.0, in1=dc["icp2"],
                    op0=SUB, op1=MULT,
                )
                if s >= 0 and s + 1 < T:
                    nc.gpsimd.tensor_tensor(
                        out=do["xat"][k2 * 32 : k2 * 32 + 32, :],
                        in0=do["att"][k2 * 32 : k2 * 32 + 32, :],
                        in1=oe_slice(o, s + 1), op=MULT,
                    )
                if s >= 0:
                    # -- o: A_new(s) = tanh(pat)
                    nc.scalar.activation(out=do["stk"][64:96, :], in_=do["patps"][bs], func=TANH)
                nc.tensor.matmul(dc["p2h"], wsb["wc"], dc["stk"][0:97, :], start=True,
                                 stop=True, tile_position=(0, 0))
                nc.scalar.activation(out=dc["mit"][bt], in_=dc["p1"][64:96, :], func=SIG)
                # hm = tanh3(h_pre) * mI
                nc.vector._custom_dve(
                    TANHMUL3_OP, out=dc["hmt"], in0=dc["p2h"], in1=dc["mit"][bt],
                    s0=c_h[0], s1=c_h[1],
                )
                # -- o: A stage for step s+1
                if s + 1 < T and s >= 0:
                    s97o = do["stk"][0:97, :]
                    nc.tensor.matmul(do["p0m"], wsb["wam"], s97o, start=True, stop=True)
                    nc.tensor.matmul(do["p0w"], wsb["waw"], s97o, start=True,
                                     stop=True, tile_position=(0, 0))
                    emit_phi(o, s)
                    nc.vector._custom_dve(
                        SIG2_OP, out=do["gmt"], in0=do["p0m"], s0=c_mW[0], s1=c_mW[1],
                    )
                nc.vector.tensor_tensor(out=dc["vt"], in0=dc["zt"][bt], in1=dc["hmt"], op=MULT)
                # I_new = v - (z-1)*I  (before xb_o: unblocks wdm -> sigma-mA)
                nc.vector.tensor_tensor(out=dc["stk"][0:32, :], in0=dc["vt"], in1=dc["un"][bt], op=SUB)
                if s + 1 < T and s >= 0:
                    xbo = do["xbt"][ks * 32 : ks * 32 + 32, :]
                    nc.vector.tensor_tensor(out=xbo, in0=do["gmt"], in1=do["p0w"], op=MULT)
                if s + 1 < T:
                    xao = do["xat"][ks * 32 : ks * 32 + 32, :]
                    xbo = do["xbt"][ks * 32 : ks * 32 + 32, :]
                    nc.vector.tensor_tensor(out=do["wpt"][(s + 1) % 2], in0=xao,
                                            in1=xbo, op=ADD)
                emit_evac(c, t - 1)

            # Warmup: both chains' step-0 W-branch inputs (xb=0, xa=attn0*oe0
            # from the bootstrap), then the steady half-period interleave.
            for h in range(NCH):
                nc.vector.tensor_tensor(out=CH[h]["wpt"][0], in0=CH[h]["xat"][0:32, :],
                                        in1=CH[h]["xbt"][0:32, :], op=ADD)
            for t in range(T):
                emit_half(0, t, 1, t - 1)
                emit_half(1, t, 0, t)
            # Tail: chain 1's C stage for the last step.
            s97o = CH[1]["stk"][0:97, :]
            nc.tensor.matmul(CH[1]["pmx"], wsb["wdm"], s97o, start=True, stop=True)
            nc.tensor.matmul(CH[1]["p3a"], wsb["wda"], s97o, start=True, stop=True,
                             tile_position=(0, 32))
            nc.scalar.activation(out=CH[1]["mat"][(T - 1) % 2][32:64, :],
                                 in_=CH[1]["pmx"][32:64, :], func=SIG)
            nc.vector.tensor_tensor(out=CH[1]["patps"][(T - 1) % 2],
                                    in0=CH[1]["mat"][(T - 1) % 2][32:64, :],
                                    in1=CH[1]["p3a"], op=MULT)
            nc.scalar.activation(out=CH[1]["stk"][64:96, :],
                                 in_=CH[1]["patps"][(T - 1) % 2], func=TANH)
            for h in range(NCH):
                emit_phi(h, T - 1)
            for h in range(NCH):
                emit_evac(h, T - 1)
    # Populate .instr bytes for InstISA subclasses (InstCustomDveAnt) —
    # raw Bass skips this Bacc pass; without it walrus codegen fails with
    # "ISA wrong length".
    mybir.codegen_inst_isa_subclasses(nc)
    return nc


_NC_CACHE = None


def kernel(**inputs):
    global _NC_CACHE
    from concourse.bass_utils import run_bass_kernel_spmd

    obs = np.ascontiguousarray(np.asarray(inputs["obs"], np.float32))
    w = _pack_weights({k: v for k, v in inputs.items() if k != "obs"})

    if _NC_CACHE is None:
        _NC_CACHE = _build_nc()
    nc = _NC_CACHE

    in_maps = []
    for i in range(N_CORES):
        m = dict(w)
        m["obs4"] = _pack_obs_shard(obs[:, i * BC : (i + 1) * BC, :])
        in_maps.append(m)

    res = run_bass_kernel_spmd(
        nc, in_maps, core_ids=list(range(N_CORES)), trace=False
    )
    outs = [_unpack_out(np.asarray(res.results[i]["out"])) for i in range(N_CORES)]
    return np.concatenate(outs, axis=1).astype(np.float32)  # [T, B, O]


# revision 43
# speedup vs baseline: 1.2068x; 1.0000x over previous
"""ANIMAZero recurrent cell on 8 TRN2 NeuronCores (Bass/Tile), v3.3.

Data-parallel: batch 1024 -> 8 cores x 128, each core running two
phase-offset chains of 64 batch columns through the T=256 recurrence.

v3 moves part of the sigma/tanh work off the ACT engine onto custom DVE
ops (clamped odd-polynomial approximations, fused with their gating
products), which relieves both the ACT fixed cost (222-cycle SBUF
access) and the two chains' ACT collisions that dominated the old
critical path:

  gmt  = SIG2(p0m)         ~ 2*sigma(mW_pre)        (waw carries the 0.5)
  rI2  = SIGMUL3(p1_r, I)  ~ 2*sigma(r_pre) * I     (wc hI rows carry 0.5)
  hm   = TANHMUL3(p2h, mI) ~ tanh(h_pre) * mI

Hardware constraints found the hard way: a DVE instruction reads at
most ONE tensor input from PSUM, and custom-DVE ops only work with ALL
operands at partition base 0. Hence the stack layout [I; W; A; ones]
(the I slot at base 0 takes the in-place custom writes), p2h/p1_r/p0m
at base 0 of their banks, and W/z/mI/attn/mA/A_new staying exact on
ACT. All biases ride the stack's ones row through the matmuls; clamp
scales are folded into the fp16 weights host-side. Polynomial
coefficients are minimax-fit against this model's empirical
pre-activation distributions (end-to-end rel err <1e-2 in simulation,
gate 2e-2).
"""

import os
import sys

sys.path.insert(0, "/opt/trn_rl_repo")
import numpy as np
import bass_rust
import concourse.bass as bass
import concourse.tile as tile
from concourse import mybir

F32 = mybir.dt.float32
F16 = mybir.dt.float16
SIG = mybir.ActivationFunctionType.Sigmoid
TANH = mybir.ActivationFunctionType.Tanh
MULT = mybir.AluOpType.mult
ADD = mybir.AluOpType.add
SUB = mybir.AluOpType.subtract

D, S, O, T, B = 32, 8, 4, 256, 1024
N_CORES = 8
BC = B // N_CORES  # 128 batch per core
G = BC
WDT = np.float16

# ---------------------------------------------------------------------------
# Polynomial approximations (t-space: t = clip(x/B, -1, 1); sigma gates
# approximate tanh(u), u = x/2). Empirically fit for this model.
PC = {
    "mW": ([0.42091229180656337, -0.023364568802614262], 0.4210716568781921),
    "r": ([0.7540762322323616, -0.11922203245848911], 0.7566058849051597),
    "h": ([1.7090931854219884, -0.8615856160238022], 1.7964036511415415),
}
B_mW, B_r, B_h = (PC[k][1] for k in ("mW", "r", "h"))

# ---------------------------------------------------------------------------
# Custom DVE ops.
from concourse.dve_spec import (  # noqa: E402
    Spec, Src0, Src1, C0, C1, C2, Zero, One, maxx, minn, sq,
    lower as dve_lower, _has_src1,
)
from concourse import dve_ops as _dvo  # noqa: E402
from concourse.dve_uop import DveOpSpec  # noqa: E402


def _register(name, body, reference):
    for o in _dvo.OPS:
        if o.name == name:
            return o
    spec = Spec(body=body, reference=reference)
    row = _dvo._CUSTOM_DVE_ROW_BASE + len(_dvo.OPS)
    assert row < 0x20
    shas = {}
    for ver in ("v3", "v4"):
        s = DveOpSpec(name=name, opcode=row, uops=dve_lower(spec, ver=ver),
                      rd1_en=_has_src1(spec))
        shas[ver] = s.sha(ver)
    op = _dvo.DveOp(name, spec, False, shas)
    _dvo.OPS.append(op)
    _dvo._SUB_OPCODE_FOR_NAME[name] = row
    _dvo.CUSTOM_DVE_SPECS[name] = spec
    return op


def _mk_ops():
    t = maxx(minn(Src0, One), Zero - One)
    t2 = sq(t)

    def np_p3(x, c0, c1):
        tt = np.clip(x, -1, 1)
        return tt * (c0 + c1 * tt * tt)

    sigmul3 = _register(
        "SIGMUL3_ANT", (One + t * (C0 + t2 * C1)) * Src1,
        lambda in0, in1, s0, s1, imm2: (1.0 + np_p3(in0, s0, s1)) * in1,
    )
    sig2 = _register(
        "SIG2_ANT", One + t * (C0 + t2 * C1),
        lambda in0, in1, s0, s1, imm2: 1.0 + np_p3(in0, s0, s1),
    )
    tanhmul3 = _register(
        "TANHMUL3_ANT", (t * (C0 + t2 * C1)) * Src1,
        lambda in0, in1, s0, s1, imm2: np_p3(in0, s0, s1) * in1,
    )
    return sigmul3, sig2, tanhmul3


SIGMUL3_OP, SIG2_OP, TANHMUL3_OP = _mk_ops()

# ---------------------------------------------------------------------------
# walrus in this container rejects instructions carrying more than one sem
# wait ("Too many sync wait commands"). After Tile lowers everything, move
# surplus waits onto same-engine NOPs inserted just before each offender.
_MAXW = 1


def _split_waits(nc):
    for f in nc.m.functions:
        for blk in f.blocks:
            il = blk.instructions
            cur = list(il)
            out_list = []
            changed = False
            for ins in cur:
                si = ins.sync_info
                w = list(si.on_wait or []) if si is not None else []
                if len(w) > _MAXW:
                    changed = True
                    for i in range(0, len(w) - _MAXW, _MAXW):
                        bi = nc.engines[ins.engine].nop(nofuse=True)
                        nop_ins = bi.ins
                        for srch in (blk,) + tuple(f.blocks):
                            lst = srch.instructions
                            if lst and lst[-1] is nop_ins:
                                lst.pop()
                                break
                        nop_ins.sync_info = bass_rust.SyncInfo(
                            on_wait=w[i : i + _MAXW], on_update=[]
                        )
                        out_list.append(nop_ins)
                    si.on_wait = w[len(w) - _MAXW :]
                out_list.append(ins)
            if changed:
                il[:] = out_list


_orig_drain = tile.TileContext._drain_and_barrier


def _drain_then_split(self, tick_clock, wait_clock):
    _orig_drain(self, tick_clock, wait_clock)
    _split_waits(self.nc)


tile.TileContext._drain_and_barrier = _drain_then_split

# ---------------------------------------------------------------------------
# Weight packing. Stack layout: rows 0:32 I, 32:64 W, 64:96 A, row 96 ones.
# All biases ride the ones row; approximation scales folded per-column.

WEIGHT_SPECS = [
    ("wam", [97, 32], F16),  # cols: mW_pre * 1/(2*B_mW)  (SIG2 approx)
    ("waw", [97, 32], F16),  # cols: W_all * 0.5  (gmt = 2*sigma)
    ("wb", [97, 96], F16),   # cols: r * 1/(2*B_r) | z | mI
    ("wc", [97, 32], F16),   # rows: hI*0.5 | hW | hA, all * 1/B_h; h_b row
    ("wdm", [97, 64], F16),  # cols: attn | mA (both exact ACT sigma)
    ("wda", [97, 32], F16),  # cols: A_all; a_b row
    ("wphi", [97, 4], F16),  # rows 64:96 = phi_w.T; row 96 = phi_b
    ("wenc", [32, 128], F16),  # 4x block-diag enc_w.T
    ("biases", [128, 8], F32),
]


def _pack_weights(inp):
    g = {k: np.ascontiguousarray(np.asarray(v, np.float32)) for k, v in inp.items()}

    def stack97(Ip, Wp, Ap, bias, scale=1.0):
        cols = next(m.shape[1] for m in (Ip, Wp, Ap) if m is not None)
        m = np.zeros((97, cols), np.float32)
        if Ip is not None:
            m[0:32] = Ip
        if Wp is not None:
            m[32:64] = Wp
        if Ap is not None:
            m[64:96] = Ap
        if bias is not None:
            m[96] = bias
        return m * scale

    # mW gate contracts [I; A] (wmg over concat(I, A))
    wam = stack97(g["wmg_w"][:, 0:32].T, None, g["wmg_w"][:, 32:64].T,
                  g["wmg_b"], 1.0 / (2.0 * B_mW))
    waw = stack97(g["wI"].T, g["wW"].T, g["wA"].T, None, 0.5)

    rc = stack97(g["rI"].T, g["rW"].T, g["rA"].T, g["r_b"], 1.0 / (2.0 * B_r))
    zc = stack97(g["zI"].T, g["zW"].T, g["zA"].T, g["z_b"])
    # mI gate contracts [W_new; A]
    mic = stack97(None, g["img_w"][:, 0:32].T, g["img_w"][:, 32:64].T, g["img_b"])
    wb = np.concatenate([rc, zc, mic], axis=1)

    wc = stack97(0.5 * g["hI"].T, g["hW"].T, g["hA"].T, g["h_b"], 1.0 / B_h)

    # attn contracts [W; I]; mA contracts [W_new; I_new]
    attc = stack97(g["att_w"][:, 32:64].T, g["att_w"][:, 0:32].T, None, g["att_b"])
    mac = stack97(g["amg_w"][:, 32:64].T, g["amg_w"][:, 0:32].T, None, g["amg_b"])
    wdm = np.concatenate([attc, mac], axis=1)

    wda = stack97(g["aI"].T, g["aW"].T, g["aA"].T, g["a_b"])

    wphi = np.zeros((97, 4), np.float32)
    wphi[64:96] = g["phi_w"].T
    wphi[96] = g["phi_b"]

    wenc = np.zeros((32, 128), np.float32)
    for k in range(4):
        wenc[k * 8 : (k + 1) * 8, k * 32 : (k + 1) * 32] = g["enc_w"].T

    biases = np.zeros((128, 8), np.float32)
    biases[0:32, 4] = g["att_b"]  # t=0 attn bootstrap
    biases[:, 5] = np.tile(g["enc_b"], 4)

    w = dict(wam=wam, waw=waw, wb=wb, wc=wc, wdm=wdm, wda=wda, wphi=wphi,
             wenc=wenc, biases=biases)
    return {
        k: np.ascontiguousarray(v if k == "biases" else v.astype(WDT))
        for k, v in w.items()
    }


def _pack_obs_shard(obs_shard):
    """[T, BC, S] f32 -> [32, T/4*BC] fp16: row k*8+s, col c*BC+b holds
    obs[4c+k, b, s] (4 timesteps stacked on partitions)."""
    x = np.ascontiguousarray(obs_shard).reshape(T // 4, 4, BC, S)
    x = x.transpose(1, 3, 0, 2)
    return np.ascontiguousarray(x.reshape(32, (T // 4) * BC)).astype(WDT)


def _unpack_out(out_core):
    """[4, T/4, 4, BC] -> [T, BC, O]."""
    return np.ascontiguousarray(
        np.asarray(out_core).reshape(O, T, BC).transpose(1, 2, 0)
    )


def _build_nc():
    nc = bass.Bass()
    obs4 = nc.declare_dram_parameter("obs4", [32, (T // 4) * BC], F16, isOutput=False)
    wdram = {}
    for name, shape, dt in WEIGHT_SPECS:
        wdram[name] = nc.declare_dram_parameter(name, shape, dt, isOutput=False)
    out = nc.declare_dram_parameter("out", [4, T // 4, 4, BC], F32, isOutput=True)

    NCH = 2
    G2 = G // NCH
    c_mW = PC["mW"][0]
    c_r = PC["r"][0]
    c_h = PC["h"][0]

    with tile.TileContext(nc) as tc:
        with (
            tc.tile_pool(name="singles", bufs=1) as singles,
            tc.tile_pool(name="psum", bufs=1, space="PSUM") as psum,
            tc.tile_pool(name="outp", bufs=3) as outp,
        ):
            wsb = {}
            for name, shape, dt in WEIGHT_SPECS:
                wsb[name] = singles.tile(shape, dt, name=f"w_{name}")
                nc.sync.dma_start(out=wsb[name], in_=wdram[name][:, :])
            obs_sb = singles.tile([32, (T // 4) * BC], F16)
            nc.sync.dma_start(out=obs_sb, in_=obs4[:, :])

            bia = wsb["biases"]

            # obs_enc_all = tanh(wenc.T @ obs4 + enc_b), all steps up front
            oenc = singles.tile([128, (T // 4) * BC], F16)
            NPRE = (T // 4) * BC // 512
            with tc.tile_pool(name="psum_pre", bufs=1, space="PSUM") as psum_pre:
                for i in range(NPRE):
                    ppre = psum_pre.tile([128, 512], F32)
                    nc.tensor.matmul(
                        ppre, wsb["wenc"], obs_sb[:, i * 512 : (i + 1) * 512],
                        start=True, stop=True,
                    )
                    nc.scalar.activation(
                        out=oenc[:, i * 512 : (i + 1) * 512], in_=ppre,
                        func=TANH, bias=bia[:, 5:6],
                    )

            def chain_tiles(h):
                d = {}
                s = lambda nm, shape: singles.tile(shape, F16, name=f"{nm}_{h}")
                d["icp2"] = s("icp2", [32, G2])  # I snapshot (Pool-written)
                d["gmt"] = s("gmt", [32, G2])    # 2*sigma(mW)  (DVE SIG2)
                d["zt"] = [s(f"zt{i}", [32, G2]) for i in range(2)]
                d["mit"] = [s(f"mit{i}", [32, G2]) for i in range(2)]  # sig(mI) @0
                d["mat"] = [s(f"mat{i}", [64, G2]) for i in range(2)]  # sig(mA) @32
                d["att"] = s("att", [128, G2])  # attn at k2*32
                d["hmt"] = s("hmt", [32, G2])
                d["vt"] = s("vt", [32, G2])
                d["un"] = [s(f"un{i}", [32, G2]) for i in range(2)]
                d["xat"] = s("xat", [128, G2])
                d["xbt"] = s("xbt", [128, G2])
                d["wpt"] = [s(f"wpt{i}", [32, G2]) for i in range(2)]
                d["stk"] = s("stk", [128, G2])
                nc.vector.memset(d["stk"], 0.0)
                nc.vector.memset(d["stk"][96:97, :], 1.0)  # ones row
                nc.vector.memset(d["icp2"], 0.0)  # I(-1) = 0
                nc.vector.memset(d["xbt"], 0.0)  # xb(0) = 0
                return d

            CH = [chain_tiles(h) for h in range(NCH)]
            for h in range(NCH):
                d = CH[h]
                tA = psum.tile([128, 2 * G2], F32, name=f"psA_{h}")
                tB = psum.tile([128, 2 * G2], F32, name=f"psB_{h}")
                tC = psum.tile([128, 7 * G2], F32, name=f"psC_{h}")
                d["pmx"] = tA[0:64, 0:G2]       # attn-pre @0:32, mA-pre @32:64
                d["p0m"] = tA[0:32, G2 : 2 * G2]
                d["p1"] = tB[0:96, 0:G2]        # r @0:32, z @32:64, mI @64:96
                d["p0w"] = tB[0:32, G2 : 2 * G2]
                d["p2h"] = tC[0:32, 0:G2]
                d["patps"] = [tC[64:96, 0:G2], tC[96:128, 0:G2]]
                d["p3a"] = tC[32:64, G2 : 2 * G2]
                d["p4"] = tC[0:4, 3 * G2 : 7 * G2]
            zz = singles.tile([32, G2], F16, name="zz")
            at0 = singles.tile([32, G2], F16, name="at0")
            nc.vector.memset(zz, 0.0)

            def emit_phi(h, t):
                # phi matmul for step t (phi_b rides the ones row), deferred
                # into step t+1's window.
                d = CH[h]
                nc.tensor.matmul(
                    d["p4"][0:4, (t % 4) * G2 : (t % 4 + 1) * G2],
                    wsb["wphi"][64:97, :], d["stk"][64:97, :],
                    start=True, stop=True,
                )

            def emit_evac(h, t):
                if t < 0 or t % 4 != 3:
                    return
                d = CH[h]
                ch = outp.tile([4, 4 * G2], F32)
                nc.scalar.copy(out=ch, in_=d["p4"])
                nc.sync.dma_start(
                    out=out[0:4, t // 4, 0:4, h * G2 : (h + 1) * G2], in_=ch
                )

            def oe_slice(h, t):
                c, k = t // 4, t % 4
                return oenc[
                    k * 32 : (k + 1) * 32,
                    c * BC + h * G2 : c * BC + (h + 1) * G2,
                ]

            # t=0 bootstrap: attn(0) = sigmoid(att_b); xa0 = attn0*oe0
            nc.scalar.activation(out=at0, in_=zz, func=SIG, bias=bia[0:32, 4:5])
            for h in range(NCH):
                nc.gpsimd.tensor_tensor(
                    out=CH[h]["xat"][0:32, :], in0=at0, in1=oe_slice(h, 0), op=MULT,
                )

            def emit_half(c, t, o, s):
                """One half-period: chain c runs its B stage for step t,
                interleaved (in ideal-schedule time order per engine) with
                chain o's C stage for step s and A stage for step s+1.
                Skip o's parts when s < 0 (warmup)."""
                dc, do = CH[c], CH[o]
                bt = t % 2
                bs = s % 2
                k2 = (s + 1) % 4
                ks = (s + 1) % 4

                # -- c: W_new(t) = tanh(wpre) (wpre built last half)
                nc.scalar.activation(out=dc["stk"][32:64, :], in_=dc["wpt"][bt], func=TANH)
                if s >= 0:
                    # -- o: C-stage matmuls for step s
                    s97o = do["stk"][0:97, :]
                    nc.tensor.matmul(do["pmx"], wsb["wdm"], s97o, start=True, stop=True)
                    nc.tensor.matmul(do["p3a"], wsb["wda"], s97o, start=True,
                                     stop=True, tile_position=(0, 32))
                    if s + 1 < T:
                        # I(s) snapshot early on Pool (ready at half start,
                        # ahead of xat in the Pool FIFO)
                        nc.gpsimd.tensor_copy(out=do["icp2"], in_=do["stk"][0:32, :])
                    nc.scalar.activation(out=do["mat"][bs][32:64, :],
                                         in_=do["pmx"][32:64, :], func=SIG)
                    nc.vector.tensor_tensor(out=do["patps"][bs],
                                            in0=do["mat"][bs][32:64, :],
                                            in1=do["p3a"], op=MULT)
                    if s + 1 < T:
                        att = do["att"][k2 * 32 : k2 * 32 + 32, :]
                        nc.scalar.activation(out=att, in_=do["pmx"][0:32, :], func=SIG)
                # -- c: B stage for step t
                nc.tensor.matmul(dc["p1"], wsb["wb"], dc["stk"][0:97, :], start=True, stop=True)
                nc.scalar.activation(out=dc["zt"][bt], in_=dc["p1"][32:64, :], func=SIG)
                # rI2 = 2*sig(r)*I overwriting the I slot (reads the Pool
                # snapshot, so not in-place); wc hI rows carry the 0.5
                nc.vector._custom_dve(
                    SIGMUL3_OP, out=dc["stk"][0:32, :], in0=dc["p1"][0:32, :],
                    in1=dc["icp2"], s0=c_r[0], s1=c_r[1],
                )
                # un = (z-1)*I in one fused DVE op (off the critical path
                # here; Pool's FIFO stalled it badly)
                nc.vector.scalar_tensor_tensor(
                    out=dc["un"][bt], in0=dc["zt"][bt], scalar=1.0, in1=dc["icp2"],
                    op0=SUB, op1=MULT,
                )
                if s >= 0 and s + 1 < T:
                    nc.gpsimd.tensor_tensor(
                        out=do["xat"][k2 * 32 : k2 * 32 + 32, :],
                        in0=do["att"][k2 * 32 : k2 * 32 + 32, :],
                        in1=oe_slice(o, s + 1), op=MULT,
                    )
                if s >= 0:
                    # -- o: A_new(s) = tanh(pat)
                    nc.scalar.activation(out=do["stk"][64:96, :], in_=do["patps"][bs], func=TANH)
                nc.tensor.matmul(dc["p2h"], wsb["wc"], dc["stk"][0:97, :], start=True,
                                 stop=True, tile_position=(0, 0))
                nc.scalar.activation(out=dc["mit"][bt], in_=dc["p1"][64:96, :], func=SIG)
                # hm = tanh3(h_pre) * mI
                nc.vector._custom_dve(
                    TANHMUL3_OP, out=dc["hmt"], in0=dc["p2h"], in1=dc["mit"][bt],
                    s0=c_h[0], s1=c_h[1],
                )
                # -- o: A stage for step s+1
                if s + 1 < T and s >= 0:
                    s97o = do["stk"][0:97, :]
                    nc.tensor.matmul(do["p0m"], wsb["wam"], s97o, start=True, stop=True)
                    nc.tensor.matmul(do["p0w"], wsb["waw"], s97o, start=True,
                                     stop=True, tile_position=(0, 0))
                    emit_phi(o, s)
                    nc.vector._custom_dve(
                        SIG2_OP, out=do["gmt"], in0=do["p0m"], s0=c_mW[0], s1=c_mW[1],
                    )
                nc.vector.tensor_tensor(out=dc["vt"], in0=dc["zt"][bt], in1=dc["hmt"], op=MULT)
                # I_new = v - (z-1)*I  (before xb_o: unblocks wdm -> sigma-mA)
                nc.vector.tensor_tensor(out=dc["stk"][0:32, :], in0=dc["vt"], in1=dc["un"][bt], op=SUB)
                if s + 1 < T and s >= 0:
                    xbo = do["xbt"][ks * 32 : ks * 32 + 32, :]
                    nc.vector.tensor_tensor(out=xbo, in0=do["gmt"], in1=do["p0w"], op=MULT)
                if s + 1 < T:
                    xao = do["xat"][ks * 32 : ks * 32 + 32, :]
                    xbo = do["xbt"][ks * 32 : ks * 32 + 32, :]
                    nc.vector.tensor_tensor(out=do["wpt"][(s + 1) % 2], in0=xao,
                                            in1=xbo, op=ADD)
                emit_evac(c, t - 1)

            # Warmup: both chains' step-0 W-branch inputs (xb=0, xa=attn0*oe0
            # from the bootstrap), then the steady half-period interleave.
            for h in range(NCH):
                nc.vector.tensor_tensor(out=CH[h]["wpt"][0], in0=CH[h]["xat"][0:32, :],
                                        in1=CH[h]["xbt"][0:32, :], op=ADD)
            for t in range(T):
                emit_half(0, t, 1, t - 1)
                emit_half(1, t, 0, t)
            # Tail: chain 1's C stage for the last step.
            s97o = CH[1]["stk"][0:97, :]
            nc.tensor.matmul(CH[1]["pmx"], wsb["wdm"], s97o, start=True, stop=True)
            nc.tensor.matmul(CH[1]["p3a"], wsb["wda"], s97o, start=True, stop=True,
                             tile_position=(0, 32))
            nc.scalar.activation(out=CH[1]["mat"][(T - 1) % 2][32:64, :],
                                 in_=CH[1]["pmx"][32:64, :], func=SIG)
            nc.vector.tensor_tensor(out=CH[1]["patps"][(T - 1) % 2],
                                    in0=CH[1]["mat"][(T - 1) % 2][32:64, :],
                                    in1=CH[1]["p3a"], op=MULT)
            nc.scalar.activation(out=CH[1]["stk"][64:96, :],
                                 in_=CH[1]["patps"][(T - 1) % 2], func=TANH)
            for h in range(NCH):
                emit_phi(h, T - 1)
            for h in range(NCH):
                emit_evac(h, T - 1)
    # Populate .instr bytes for InstISA subclasses (InstCustomDveAnt) —
    # raw Bass skips this Bacc pass; without it walrus codegen fails with
    # "ISA wrong length".
    mybir.codegen_inst_isa_subclasses(nc)
    return nc


_NC_CACHE = None


def kernel(**inputs):
    global _NC_CACHE
    from concourse.bass_utils import run_bass_kernel_spmd

    obs = np.ascontiguousarray(np.asarray(inputs["obs"], np.float32))
    w = _pack_weights({k: v for k, v in inputs.items() if k != "obs"})

    if _NC_CACHE is None:
        _NC_CACHE = _build_nc()
    nc = _NC_CACHE

    in_maps = []
    for i in range(N_CORES):
        m = dict(w)
        m["obs4"] = _pack_obs_shard(obs[:, i * BC : (i + 1) * BC, :])
        in_maps.append(m)

    res = run_bass_kernel_spmd(
        nc, in_maps, core_ids=list(range(N_CORES)), trace=False
    )
    outs = [_unpack_out(np.asarray(res.results[i]["out"])) for i in range(N_CORES)]
    return np.concatenate(outs, axis=1).astype(np.float32)  # [T, B, O]
